# revision 3
# baseline (speedup 1.0000x reference)
"""Trainium2 Bass kernel for ContinuousREWAEncoder:
    out = FWHT(x @ W^T)/sqrt(32) + 0.01*normal(key=42)

Math folding: FWHT is linear => out = x @ (H @ W / sqrt(32))^T + noise.
The noise uses a fixed PRNG key, so it is a deterministic constant computed
on host and added on host to the device result (device output is fp16, so
the host add costs ~5e-4 relative error -- negligible vs the fp8 x quant).

Sharding: pure data parallel over tokens (B*N = 32768 -> 4096/core on 8
cores). W_eff is replicated.

The kernel is HBM-bandwidth bound, so traffic is minimized:
  - x is quantized on host to fp8 e3m4 (1 B/elem; measured end-to-end
    maxrel ~1.1e-2 vs the fp32 reference, threshold 2e-2). W stays fp16
    (mixed-dtype PE matmul; both upconvert internally).
  - out is fp16, packed to a full 128-partition layout [m + 32q, j]
    (q = token//1024, j = token%1024) so stores use all SBUF partitions.
W is placed at PE column-group q = block//2 via tile_position=(0,32q), so
each block's [32,512] result lands at PSUM partitions 32q..32q+32 --
aligned with the packed out partition base. Blocks are processed in PAIRS
with the two blocks' matmuls interleaved chunk-by-chunk: the pair uses
two different PE column groups, so the matmuls overlap in disjoint
32-column strips of the PE array (~2x effective matmul rate).
x DMAs are split across both HWDGE rings (sync + scalar) to halve the
issue-serialization ramp. Pair order (0,2),(4,6),(1,3),(5,7) lets the
out columns [0:512) store at ~60% and [512:1024) at the end.
"""

import math

import numpy as np
import ml_dtypes

import concourse.tile as tile
from concourse import bacc, mybir
from concourse.bass_utils import run_bass_kernel_spmd

B, N, D, M = 4, 8192, 1024, 32
NOISE_STD = 0.01
N_CORES = 8
TOK_TOTAL = B * N              # 32768
TOK = TOK_TOTAL // N_CORES     # 4096 tokens per core
BLK = 512                      # tokens per PSUM bank ([32, 512] fp32)
NBLK = TOK // BLK              # 8 blocks -> the 8 PSUM banks
KC = D // 128                  # 8 contraction chunks
QTOK = TOK // 4                # 1024 tokens per column-group quarter

X_DT = mybir.dt.float8e3
X_NP = ml_dtypes.float8_e3m4
W_DT = mybir.dt.float16
F16 = mybir.dt.float16
F32 = mybir.dt.float32

# block pairs, processed with chunk-interleaved matmuls; evens first so
# the [0:512) column store can fire early
PAIRS = [(0, 2), (4, 6), (1, 3), (5, 7)]
ORDER = [b for p in PAIRS for b in p]          # DRAM layout order of x blocks
# ring assignment for x block DMAs: 'a' = sync, 'b' = scalar (W + rest)
SYNC_BLOCKS = [0, 4, 1, 5]
SCAL_BLOCKS = [2, 6, 3, 7]


def _build_bass():
    nc = bacc.Bacc("TRN2", target_bir_lowering=False)

    # x pre-tiled on host to [proc_idx, partition, kchunk*BLK] fp8 so each
    # DMA moves one contiguous 4 KB run per partition. First axis ordered
    # by ORDER (processing order).
    xT = nc.dram_tensor("xT", [NBLK, 128, KC * BLK], X_DT, kind="ExternalInput")
    wT = nc.dram_tensor("wT", [128, KC * M], W_DT, kind="ExternalInput")
    outT = nc.dram_tensor("outT", [128, QTOK], F16, kind="ExternalOutput")

    dram_idx = {b: i for i, b in enumerate(ORDER)}

    with tile.TileContext(nc) as tc:
        with (
            tc.tile_pool(name="w", bufs=1) as wpool,
            tc.tile_pool(name="x", bufs=7) as xpool,
            tc.tile_pool(name="x0", bufs=2) as x0pool,
            tc.tile_pool(name="out", bufs=1) as opool,
            tc.tile_pool(name="psum", bufs=8, space="PSUM") as ppool,
        ):
            # w on the scalar ring (parallel with the first x DMA on sync).
            w_tile = wpool.tile([128, KC, M], W_DT)
            nc.scalar.dma_start(w_tile[:], wT.rearrange("p (c m) -> p c m", c=KC))

            # x DMAs, split across the two HWDGE rings. Block 0 is halved
            # so the first matmuls start after 256 KB instead of 512 KB.
            x_slices = {}  # block -> list of (chunk_lo, chunk_hi, tile)
            for b_sync, b_scal in zip(SYNC_BLOCKS, SCAL_BLOCKS):
                for b, eng in ((b_sync, nc.sync), (b_scal, nc.scalar)):
                    xsrc = xT[dram_idx[b]].rearrange("p (c t) -> p c t", c=KC)
                    if b == 0:
                        ta = x0pool.tile([128, 4, BLK], X_DT, tag="x0")
                        eng.dma_start(ta[:], xsrc[:, 0:4, :])
                        tb = x0pool.tile([128, 4, BLK], X_DT, tag="x0")
                        eng.dma_start(tb[:], xsrc[:, 4:8, :])
                        x_slices[b] = [(0, 4, ta), (4, 8, tb)]
                    else:
                        t = xpool.tile([128, KC, BLK], X_DT, tag="xt")
                        eng.dma_start(t[:], xsrc)
                        x_slices[b] = [(0, KC, t)]

            def chunk_tile(b, c):
                for lo, hi, t in x_slices[b]:
                    if lo <= c < hi:
                        return t[:, c - lo, :]
                raise AssertionError

            # Warmup matmul: absorbs the w-DMA wait into PE program order
            # so every real matmul needs only its x-DMA sync wait.
            warm = ppool.tile([M, M], F32, tag="ptile")
            nc.tensor.matmul(warm[:], w_tile[:, 0, :], w_tile[:, 0, :])

            out_sb = opool.tile([128, QTOK], F16)

            for pi, (ba, bb) in enumerate(PAIRS):
                tiles = {}
                for b in (ba, bb):
                    q = b // 2
                    ptile = ppool.tile([128, BLK], F32, tag="ptile")
                    tiles[b] = (q, ptile[32 * q : 32 * q + 32, :])
                for c in range(KC):
                    for b in (ba, bb):
                        q, pslice = tiles[b]
                        nc.tensor.matmul(
                            pslice,
                            w_tile[:, c, :],
                            chunk_tile(b, c),
                            start=(c == 0),
                            stop=(c == KC - 1),
                            tile_position=(0, 32 * q),
                        )
                for b in (ba, bb):
                    q, pslice = tiles[b]
                    col = (b % 2) * BLK
                    nc.vector.tensor_copy(
                        out_sb[32 * q : 32 * q + 32, col : col + BLK], pslice
                    )
                if pi == 1:
                    nc.sync.dma_start(outT[:, 0:BLK], out_sb[:, 0:BLK])
                elif pi == 3:
                    nc.sync.dma_start(outT[:, BLK:QTOK], out_sb[:, BLK:QTOK])

    nc.compile()
    return nc


_NC_CACHE = None


def _get_nc():
    global _NC_CACHE
    if _NC_CACHE is None:
        _NC_CACHE = _build_bass()
    return _NC_CACHE


def _hadamard32() -> np.ndarray:
    h = np.array([[1.0]], dtype=np.float64)
    while h.shape[0] < M:
        h = np.block([[h, h], [h, -h]])
    return h


_NOISE_CACHE = None


def _noise() -> np.ndarray:
    # Mirror reference.py exactly (same op on the default jax backend): the
    # bits differ between backends, so the noise must be produced the same
    # way the grading reference produces it.
    global _NOISE_CACHE
    if _NOISE_CACHE is None:
        import jax

        nz = NOISE_STD * jax.random.normal(
            jax.random.key(42), (B, N, M), dtype=np.float32
        )
        _NOISE_CACHE = np.asarray(nz)
    return _NOISE_CACHE


def kernel(x: np.ndarray, W: np.ndarray, _profile_sink=None) -> np.ndarray:
    x = np.ascontiguousarray(np.asarray(x, dtype=np.float32))
    W = np.asarray(W, dtype=np.float32)

    # Fold normalized FWHT into the projection: out = x @ w_lhsT + noise
    w_eff = (_hadamard32() @ W.astype(np.float64)) / math.sqrt(M)
    w_lhsT = w_eff.T.astype(np.float16)  # [D, M]
    w_dev = np.ascontiguousarray(
        w_lhsT.reshape(KC, 128, M).transpose(1, 0, 2)
    ).reshape(128, KC * M)

    X = x.reshape(TOK_TOTAL, D).astype(X_NP)

    in_maps = []
    for i in range(N_CORES):
        sl = slice(i * TOK, (i + 1) * TOK)
        # [tok, d] -> [blk, partition, kchunk, tok_in_blk], blocks in ORDER
        xt = np.ascontiguousarray(
            X[sl].reshape(NBLK, BLK, KC, 128).transpose(0, 3, 2, 1)[ORDER]
        ).reshape(NBLK, 128, KC * BLK)
        in_maps.append({"xT": xt, "wT": w_dev})

    res = run_bass_kernel_spmd(
        _get_nc(),
        in_maps,
        core_ids=list(range(N_CORES)),
        trace=_profile_sink is not None,
    )
    if _profile_sink is not None:
        _profile_sink.append(res)

    # unpack [m + 32q, j] -> [tok, m], then add noise on host in fp32
    outs = []
    for r in res.results:
        o = r["outT"].reshape(4, M, QTOK).transpose(0, 2, 1).reshape(TOK, M)
        outs.append(o)
    out = np.concatenate(outs, axis=0).astype(np.float32)
    out += _noise().reshape(TOK_TOTAL, M)
    return np.ascontiguousarray(out.reshape(B, N, M))


if __name__ == "__main__":
    xs = np.random.randn(B, N, D).astype(np.float32)
    Ws = (np.random.randn(M, D) / math.sqrt(D)).astype(np.float32)
    o = kernel(xs, Ws)
    print(o.shape, o.dtype)


# revision 4
# speedup vs baseline: 1.0637x; 1.0637x over previous
"""Trainium2 Bass kernel for ContinuousREWAEncoder:
    out = FWHT(x @ W^T)/sqrt(32) + 0.01*normal(key=42)

Math folding: FWHT is linear => out = x @ (H @ W / sqrt(32))^T + noise.
The noise uses a fixed PRNG key, so it is a deterministic constant computed
on host and added on host to the device result (device output is fp16, so
the host add costs ~5e-4 relative error -- negligible vs the fp8 x quant).

Sharding: pure data parallel over tokens (B*N = 32768 -> 4096/core on 8
cores). W_eff is replicated.

The kernel is HBM-bandwidth bound, so traffic is minimized:
  - x is quantized on host to fp8 e3m4 (1 B/elem; measured end-to-end
    maxrel ~1.1e-2 vs the fp32 reference, threshold 2e-2). W stays fp16
    (mixed-dtype PE matmul; both upconvert internally).
  - out is fp16, packed to a full 128-partition layout [m + 32q, j]
    (q = token//1024, j = token%1024) so stores use all SBUF partitions.
W is placed at PE column-group q = block//2 via tile_position=(0,32q), so
each block's [32,512] result lands at PSUM partitions 32q..32q+32 --
aligned with the packed out partition base. Blocks are processed in PAIRS
with the two blocks' matmuls interleaved chunk-by-chunk: the pair uses
two different PE column groups, so the matmuls overlap in disjoint
32-column strips of the PE array (~2x effective matmul rate).
x DMAs are split across both HWDGE rings (sync + scalar) to halve the
issue-serialization ramp. Pair order (0,2),(4,6),(1,3),(5,7) lets the
out columns [0:512) store at ~60% and [512:1024) at the end.
"""

import math

import numpy as np
import ml_dtypes

import concourse.tile as tile
from concourse import bacc, mybir
from concourse.bass_utils import run_bass_kernel_spmd

B, N, D, M = 4, 8192, 1024, 32
NOISE_STD = 0.01
N_CORES = 8
TOK_TOTAL = B * N              # 32768
TOK = TOK_TOTAL // N_CORES     # 4096 tokens per core
BLK = 512                      # tokens per PSUM bank ([32, 512] fp32)
NBLK = TOK // BLK              # 8 blocks -> the 8 PSUM banks
KC = D // 128                  # 8 contraction chunks
QTOK = TOK // 4                # 1024 tokens per column-group quarter

X_DT = mybir.dt.float8e3
X_NP = ml_dtypes.float8_e3m4
W_DT = mybir.dt.float16
F16 = mybir.dt.float16
F32 = mybir.dt.float32

# Processing (and DRAM layout) order of x blocks: evens first so the
# [0:512) column store can fire at ~50%, and adjacent blocks alternate PE
# column groups (q = b//2 -> 0,1,2,3,0,1,2,3) so the sliding interleave
# below overlaps matmuls of consecutive blocks in disjoint PE columns.
ORDER = [0, 2, 4, 6, 1, 3, 5, 7]


def _build_bass():
    nc = bacc.Bacc("TRN2", target_bir_lowering=False)

    # x pre-tiled on host to [proc_idx, partition, kchunk*BLK] fp8 so each
    # DMA moves one contiguous 4 KB run per partition. First axis ordered
    # by ORDER (processing order).
    xT = nc.dram_tensor("xT", [NBLK, 128, KC * BLK], X_DT, kind="ExternalInput")
    wT = nc.dram_tensor("wT", [128, KC * M], W_DT, kind="ExternalInput")
    outT = nc.dram_tensor("outT", [128, QTOK], F16, kind="ExternalOutput")

    with tile.TileContext(nc) as tc:
        with (
            tc.tile_pool(name="w", bufs=1) as wpool,
            tc.tile_pool(name="x", bufs=7) as xpool,
            tc.tile_pool(name="x0", bufs=2) as x0pool,
            tc.tile_pool(name="out", bufs=1) as opool,
            tc.tile_pool(name="psum", bufs=8, space="PSUM") as ppool,
        ):
            # w on the scalar ring (parallel with the x stream on sync).
            w_tile = wpool.tile([128, KC, M], W_DT)
            nc.scalar.dma_start(w_tile[:], wT.rearrange("p (c m) -> p c m", c=KC))

            # All x on the sync ring, in processing order (two interleaved
            # rings measured ~20% lower aggregate HBM rate). Block 0 is
            # halved so the first matmuls start after 256 KB.
            x_slices = {}  # block -> list of (chunk_lo, chunk_hi, tile)
            for i, b in enumerate(ORDER):
                xsrc = xT[i].rearrange("p (c t) -> p c t", c=KC)
                if b == 0:
                    ta = x0pool.tile([128, 4, BLK], X_DT, tag="x0")
                    nc.sync.dma_start(ta[:], xsrc[:, 0:4, :])
                    tb = x0pool.tile([128, 4, BLK], X_DT, tag="x0")
                    nc.sync.dma_start(tb[:], xsrc[:, 4:8, :])
                    x_slices[b] = [(0, 4, ta), (4, 8, tb)]
                else:
                    t = xpool.tile([128, KC, BLK], X_DT, tag="xt")
                    nc.sync.dma_start(t[:], xsrc)
                    x_slices[b] = [(0, KC, t)]

            def chunk_tile(b, c):
                for lo, hi, t in x_slices[b]:
                    if lo <= c < hi:
                        return t[:, c - lo, :]
                raise AssertionError

            # Warmup matmul: absorbs the w-DMA wait into PE program order
            # so every real matmul needs only its x-DMA sync wait.
            warm = ppool.tile([M, M], F32, tag="ptile")
            nc.tensor.matmul(warm[:], w_tile[:, 0, :], w_tile[:, 0, :])

            out_sb = opool.tile([128, QTOK], F16)

            psl = {}
            for b in ORDER:
                q = b // 2
                ptile = ppool.tile([128, BLK], F32, tag="ptile")
                psl[b] = ptile[32 * q : 32 * q + 32, :]

            def mm(b, c):
                nc.tensor.matmul(
                    psl[b],
                    w_tile[:, c, :],
                    chunk_tile(b, c),
                    start=(c == 0),
                    stop=(c == KC - 1),
                    tile_position=(0, 32 * (b // 2)),
                )

            def evac(b):
                col = (b % 2) * BLK
                q = b // 2
                nc.vector.tensor_copy(
                    out_sb[32 * q : 32 * q + 32, col : col + BLK], psl[b]
                )

            # Sliding interleave: block A's chunks 4-7 interleave with the
            # next block B's chunks 0-3; A and B are in different PE column
            # groups so these matmuls overlap in the array. Each matmul
            # waits only on its own block's (already streaming) DMA.
            half = KC // 2
            for c in range(half):
                mm(ORDER[0], c)
            for i in range(NBLK - 1):
                a, b = ORDER[i], ORDER[i + 1]
                for c in range(half):
                    mm(a, half + c)
                    mm(b, c)
                evac(a)
                if a == 6:  # blocks 0,2,4,6 done -> store packed cols [0:512)
                    nc.sync.dma_start(outT[:, 0:BLK], out_sb[:, 0:BLK])
            last = ORDER[-1]
            for c in range(half):
                mm(last, half + c)
            evac(last)
            nc.sync.dma_start(outT[:, BLK:QTOK], out_sb[:, BLK:QTOK])

    nc.compile()
    return nc


_NC_CACHE = None


def _get_nc():
    global _NC_CACHE
    if _NC_CACHE is None:
        _NC_CACHE = _build_bass()
    return _NC_CACHE


def _hadamard32() -> np.ndarray:
    h = np.array([[1.0]], dtype=np.float64)
    while h.shape[0] < M:
        h = np.block([[h, h], [h, -h]])
    return h


_NOISE_CACHE = None


def _noise() -> np.ndarray:
    # Mirror reference.py exactly (same op on the default jax backend): the
    # bits differ between backends, so the noise must be produced the same
    # way the grading reference produces it.
    global _NOISE_CACHE
    if _NOISE_CACHE is None:
        import jax

        nz = NOISE_STD * jax.random.normal(
            jax.random.key(42), (B, N, M), dtype=np.float32
        )
        _NOISE_CACHE = np.asarray(nz)
    return _NOISE_CACHE


def kernel(x: np.ndarray, W: np.ndarray, _profile_sink=None) -> np.ndarray:
    x = np.ascontiguousarray(np.asarray(x, dtype=np.float32))
    W = np.asarray(W, dtype=np.float32)

    # Fold normalized FWHT into the projection: out = x @ w_lhsT + noise
    w_eff = (_hadamard32() @ W.astype(np.float64)) / math.sqrt(M)
    w_lhsT = w_eff.T.astype(np.float16)  # [D, M]
    w_dev = np.ascontiguousarray(
        w_lhsT.reshape(KC, 128, M).transpose(1, 0, 2)
    ).reshape(128, KC * M)

    X = x.reshape(TOK_TOTAL, D).astype(X_NP)

    in_maps = []
    for i in range(N_CORES):
        sl = slice(i * TOK, (i + 1) * TOK)
        # [tok, d] -> [blk, partition, kchunk, tok_in_blk], blocks in ORDER
        xt = np.ascontiguousarray(
            X[sl].reshape(NBLK, BLK, KC, 128).transpose(0, 3, 2, 1)[ORDER]
        ).reshape(NBLK, 128, KC * BLK)
        in_maps.append({"xT": xt, "wT": w_dev})

    res = run_bass_kernel_spmd(
        _get_nc(),
        in_maps,
        core_ids=list(range(N_CORES)),
        trace=_profile_sink is not None,
    )
    if _profile_sink is not None:
        _profile_sink.append(res)

    # unpack [m + 32q, j] -> [tok, m], then add noise on host in fp32
    outs = []
    for r in res.results:
        o = r["outT"].reshape(4, M, QTOK).transpose(0, 2, 1).reshape(TOK, M)
        outs.append(o)
    out = np.concatenate(outs, axis=0).astype(np.float32)
    out += _noise().reshape(TOK_TOTAL, M)
    return np.ascontiguousarray(out.reshape(B, N, M))


if __name__ == "__main__":
    xs = np.random.randn(B, N, D).astype(np.float32)
    Ws = (np.random.randn(M, D) / math.sqrt(D)).astype(np.float32)
    o = kernel(xs, Ws)
    print(o.shape, o.dtype)


# revision 6
# speedup vs baseline: 1.1142x; 1.0474x over previous
"""Trainium2 Bass kernel for ContinuousREWAEncoder:
    out = FWHT(x @ W^T)/sqrt(32) + 0.01*normal(key=42)

Math folding: FWHT is linear => out = x @ (H @ W / sqrt(32))^T + noise.
The noise uses a fixed PRNG key, so it is a deterministic constant computed
on host and added on host to the device result (device output is fp16, so
the host add costs ~5e-4 relative error -- negligible vs the fp8 x quant).

Sharding: pure data parallel over tokens (B*N = 32768 -> 4096/core on 8
cores). W_eff is replicated.

The kernel is HBM-bandwidth bound, so traffic is minimized:
  - x is quantized on host to fp8 e3m4 (1 B/elem; measured end-to-end
    maxrel ~1.1e-2 vs the fp32 reference, threshold 2e-2). W stays fp16
    (mixed-dtype PE matmul; both upconvert internally).
  - out is fp16, packed to a full 128-partition layout [m + 32q, j]
    (q = token//1024, j = token%1024) so stores use all SBUF partitions.
W is placed at PE column-group q = block//2 via tile_position=(0,32q), so
each block's [32,512] result lands at PSUM partitions 32q..32q+32 --
aligned with the packed out partition base. Blocks are processed in PAIRS
with the two blocks' matmuls interleaved chunk-by-chunk: the pair uses
two different PE column groups, so the matmuls overlap in disjoint
32-column strips of the PE array (~2x effective matmul rate).
x DMAs are split across both HWDGE rings (sync + scalar) to halve the
issue-serialization ramp. Pair order (0,2),(4,6),(1,3),(5,7) lets the
out columns [0:512) store at ~60% and [512:1024) at the end.
"""

import math

import numpy as np
import ml_dtypes

import concourse.tile as tile
from concourse import bacc, mybir
from concourse.bass_utils import run_bass_kernel_spmd

B, N, D, M = 4, 8192, 1024, 32
NOISE_STD = 0.01
N_CORES = 8
TOK_TOTAL = B * N              # 32768
TOK = TOK_TOTAL // N_CORES     # 4096 tokens per core
BLK = 512                      # tokens per PSUM bank ([32, 512] fp32)
NBLK = TOK // BLK              # 8 blocks -> the 8 PSUM banks
KC = D // 128                  # 8 contraction chunks
QTOK = TOK // 4                # 1024 tokens per column-group quarter

X_DT = mybir.dt.float8e3
X_NP = ml_dtypes.float8_e3m4
W_DT = mybir.dt.float16
F16 = mybir.dt.float16
F32 = mybir.dt.float32

# Processing (and DRAM layout) order of x blocks: evens first so the
# [0:512) column store can fire at ~50%, and adjacent blocks alternate PE
# column groups (q = b//2 -> 0,1,2,3,0,1,2,3) so the sliding interleave
# below overlaps matmuls of consecutive blocks in disjoint PE columns.
ORDER = [0, 2, 4, 6, 1, 3, 5, 7]


def _build_bass():
    nc = bacc.Bacc("TRN2", target_bir_lowering=False)

    # x pre-tiled on host to [proc_idx, partition, kchunk*BLK] fp8 so each
    # DMA moves one contiguous 4 KB run per partition. First axis ordered
    # by ORDER (processing order).
    xT = nc.dram_tensor("xT", [NBLK, 128, KC * BLK], X_DT, kind="ExternalInput")
    wT = nc.dram_tensor("wT", [128, KC * M], W_DT, kind="ExternalInput")
    outT = nc.dram_tensor("outT", [128, QTOK], F16, kind="ExternalOutput")

    with tile.TileContext(nc) as tc:
        with (
            tc.tile_pool(name="w", bufs=1) as wpool,
            tc.tile_pool(name="x", bufs=6) as xpool,
            tc.tile_pool(name="x0", bufs=3) as x0pool,
            tc.tile_pool(name="x7", bufs=3) as x7pool,
            tc.tile_pool(name="out", bufs=1) as opool,
            tc.tile_pool(name="psum", bufs=8, space="PSUM") as ppool,
        ):
            # w on the scalar ring (parallel with the x stream on sync).
            w_tile = wpool.tile([128, KC, M], W_DT)
            nc.scalar.dma_start(w_tile[:], wT.rearrange("p (c m) -> p c m", c=KC))

            # All x on the sync ring, in processing order (two interleaved
            # bulk rings measured ~20% lower aggregate HBM rate). The first
            # processed block is split 2/2/4 chunks so the first matmuls
            # start as early as possible; the last is split 4/2/2 so the
            # final matmuls wait only on the last 64 KB.
            x_slices = {}  # block -> list of (chunk_lo, chunk_hi, tile)

            def xdma(i, b, lo, hi, pool, tag):
                xsrc = xT[i].rearrange("p (c t) -> p c t", c=KC)
                t = pool.tile([128, hi - lo, BLK], X_DT, tag=tag)
                nc.sync.dma_start(t[:], xsrc[:, lo:hi, :])
                x_slices.setdefault(b, []).append((lo, hi, t))

            first, last = ORDER[0], ORDER[-1]
            ilast = NBLK - 1
            xdma(0, first, 0, 2, x0pool, "x0")
            xdma(0, first, 2, 4, x0pool, "x0")
            xdma(1, ORDER[1], 0, KC, xpool, "xt")
            xdma(0, first, 4, 8, x0pool, "x0")
            for i in range(2, ilast):
                xdma(i, ORDER[i], 0, KC, xpool, "xt")
            xdma(ilast, last, 0, 4, x7pool, "x7")
            xdma(ilast, last, 4, 6, x7pool, "x7")
            xdma(ilast, last, 6, 8, x7pool, "x7")

            def chunk_tile(b, c):
                for lo, hi, t in x_slices[b]:
                    if lo <= c < hi:
                        return t[:, c - lo, :]
                raise AssertionError

            # Warmup matmul: absorbs the w-DMA wait into PE program order
            # so every real matmul needs only its x-DMA sync wait.
            warm = ppool.tile([M, M], F32, tag="ptile")
            nc.tensor.matmul(warm[:], w_tile[:, 0, :], w_tile[:, 0, :])

            out_sb = opool.tile([128, QTOK], F16)

            psl = {}
            for b in ORDER:
                q = b // 2
                ptile = ppool.tile([128, BLK], F32, tag="ptile")
                psl[b] = ptile[32 * q : 32 * q + 32, :]

            def mm(b, c):
                nc.tensor.matmul(
                    psl[b],
                    w_tile[:, c, :],
                    chunk_tile(b, c),
                    start=(c == 0),
                    stop=(c == KC - 1),
                    tile_position=(0, 32 * (b // 2)),
                )

            def evac(b):
                col = (b % 2) * BLK
                q = b // 2
                nc.vector.tensor_copy(
                    out_sb[32 * q : 32 * q + 32, col : col + BLK], psl[b]
                )

            # Sliding interleave: block A's chunks 4-7 interleave with the
            # next block B's chunks 0-3; A and B are in different PE column
            # groups so these matmuls overlap in the array. Each matmul
            # waits only on its own block's (already streaming) DMA.
            half = KC // 2
            for c in range(half):
                mm(first, c)
            for i in range(NBLK - 1):
                a, b = ORDER[i], ORDER[i + 1]
                for c in range(half):
                    mm(a, half + c)
                    mm(b, c)
                evac(a)
                if a == 6:  # blocks 0,2,4,6 done -> store packed cols [0:512)
                    nc.sync.dma_start(outT[:, 0:BLK], out_sb[:, 0:BLK])
                elif a == 5:  # blocks 1,3,5 done -> store rows [0:96)
                    nc.sync.dma_start(
                        outT[0:96, BLK:QTOK], out_sb[0:96, BLK:QTOK]
                    )
            for c in range(half):
                mm(last, half + c)
            evac(last)
            nc.sync.dma_start(outT[96:128, BLK:QTOK], out_sb[96:128, BLK:QTOK])

    nc.compile()
    return nc


_NC_CACHE = None


def _get_nc():
    global _NC_CACHE
    if _NC_CACHE is None:
        _NC_CACHE = _build_bass()
    return _NC_CACHE


def _hadamard32() -> np.ndarray:
    h = np.array([[1.0]], dtype=np.float64)
    while h.shape[0] < M:
        h = np.block([[h, h], [h, -h]])
    return h


_NOISE_CACHE = None


def _noise() -> np.ndarray:
    # Mirror reference.py exactly (same op on the default jax backend): the
    # bits differ between backends, so the noise must be produced the same
    # way the grading reference produces it.
    global _NOISE_CACHE
    if _NOISE_CACHE is None:
        import jax

        nz = NOISE_STD * jax.random.normal(
            jax.random.key(42), (B, N, M), dtype=np.float32
        )
        _NOISE_CACHE = np.asarray(nz)
    return _NOISE_CACHE


def kernel(x: np.ndarray, W: np.ndarray, _profile_sink=None) -> np.ndarray:
    x = np.ascontiguousarray(np.asarray(x, dtype=np.float32))
    W = np.asarray(W, dtype=np.float32)

    # Fold normalized FWHT into the projection: out = x @ w_lhsT + noise
    w_eff = (_hadamard32() @ W.astype(np.float64)) / math.sqrt(M)
    w_lhsT = w_eff.T.astype(np.float16)  # [D, M]
    w_dev = np.ascontiguousarray(
        w_lhsT.reshape(KC, 128, M).transpose(1, 0, 2)
    ).reshape(128, KC * M)

    X = x.reshape(TOK_TOTAL, D).astype(X_NP)

    in_maps = []
    for i in range(N_CORES):
        sl = slice(i * TOK, (i + 1) * TOK)
        # [tok, d] -> [blk, partition, kchunk, tok_in_blk], blocks in ORDER
        xt = np.ascontiguousarray(
            X[sl].reshape(NBLK, BLK, KC, 128).transpose(0, 3, 2, 1)[ORDER]
        ).reshape(NBLK, 128, KC * BLK)
        in_maps.append({"xT": xt, "wT": w_dev})

    res = run_bass_kernel_spmd(
        _get_nc(),
        in_maps,
        core_ids=list(range(N_CORES)),
        trace=_profile_sink is not None,
    )
    if _profile_sink is not None:
        _profile_sink.append(res)

    # unpack [m + 32q, j] -> [tok, m], then add noise on host in fp32
    outs = []
    for r in res.results:
        o = r["outT"].reshape(4, M, QTOK).transpose(0, 2, 1).reshape(TOK, M)
        outs.append(o)
    out = np.concatenate(outs, axis=0).astype(np.float32)
    out += _noise().reshape(TOK_TOTAL, M)
    return np.ascontiguousarray(out.reshape(B, N, M))


if __name__ == "__main__":
    xs = np.random.randn(B, N, D).astype(np.float32)
    Ws = (np.random.randn(M, D) / math.sqrt(D)).astype(np.float32)
    o = kernel(xs, Ws)
    print(o.shape, o.dtype)
